# revision 7
# baseline (speedup 1.0000x reference)
"""Trainium2 Bass kernel for DifferentialMultiHeadCrossAttention.

Sharding (8 cores): 2-way data parallel over batch x 4-way head parallel
(4 heads per core).  Each core computes, for its batch b and its 4 heads,
the full attention + its row-slice of the output projection Wo, producing a
partial [T, E] output.  The host sums the 4 partials per batch (row-parallel
all-reduce done at unshard time) and adds bo.

Per-core algorithm (T=2048, E=256, DH=256, dh2=128), all fp32 (fp32r matmuls):
  xT, encT  = transpose(x), transpose(enc)               (PE transposes, once)
  per head h:
    qT[d,t] = Wq[h].T @ x.T   (+bq folded into PSUM->SBUF copy bias)
    kT[d,s] = Wk[h].T @ enc.T (+bk)
    v[s,d]  = enc @ Wv[h]     (+bv) ; v_aug = [v | ones]  (N=257)
    per 256-wide t-block:
      per s-chunk (16): sT1|sT2 [s128, t256|t256] = k.q matmuls -> PSUM
                        e12 = exp(scale * sT12)          (ACT, PSUM->SBUF)
      per t128 chunk:   o1 = e1T.T @ v_aug, o2 = e2T.T @ v_aug (PSUM acc over s)
                        o1[:,256] = Z1, o2[:,256] = Z2 (softmax denominators)
        combine: oc = o1/Z1 - lam*o2/Z2   (DVE, per-partition scalars)
        ssq[t] = sum_d oc^2               (fused tensor_tensor_reduce)
    rinv = exp(-0.5*ln(ssq/256 + eps))   (ACT, same table set as Exp)
    per t128: oc *= rinv ; oNT = PE-transpose(oc) * g*(1-lam0) (fold in copy)
              partial[t,e] += oNT.T @ Wo[h]  (accumulated across heads on DVE)
"""

import sys

for _p in ("/opt/trn_rl_repo",):
    if _p not in sys.path:
        sys.path.insert(0, _p)

import numpy as np

import concourse.bass as bass
import concourse.mybir as mybir
import concourse.tile as tile
from concourse import bacc
from concourse.bass_utils import run_bass_kernel_spmd
from concourse.masks import make_identity

F32 = mybir.dt.float32
F32R = mybir.dt.float32r
AF = mybir.ActivationFunctionType
ALU = mybir.AluOpType

B, T, E, H, DH = 2, 2048, 256, 16, 256
DH2 = DH // 2  # 128
N_CORES = 8
HPC = H // 4  # heads per core = 4
LAMBDA_INIT = 0.8
EPS = 1.1920929e-07
SCALE = float(1.0 / np.sqrt(np.float32(DH2)))

TBLK = 256          # t-block width for the score/exp pipeline
NTB = None          # set from seq len in build
SCHUNK = 128        # s chunk (partition dim of transposed scores)


def build_program(seq=T, nheads=HPC):
    """Build the SPMD per-core Bass program. Returns nc."""
    nsc = seq // SCHUNK          # number of s chunks
    ntb = seq // TBLK            # number of 256-wide t blocks
    nt128 = seq // 128           # number of 128-wide t chunks
    nt512 = seq // 512           # 512-wide t blocks for qk projections

    nc = bacc.Bacc("TRN2", target_bir_lowering=False, debug=False,
                   num_devices=N_CORES)

    x_d = nc.dram_tensor("x", [seq, E], F32, kind="ExternalInput")
    enc_d = nc.dram_tensor("enc", [seq, E], F32, kind="ExternalInput")
    wq_d = nc.dram_tensor("wq", [nheads, E, DH], F32, kind="ExternalInput")
    wk_d = nc.dram_tensor("wk", [nheads, E, DH], F32, kind="ExternalInput")
    wv_d = nc.dram_tensor("wv", [nheads, E, DH], F32, kind="ExternalInput")
    wo_d = nc.dram_tensor("wo", [nheads, DH, E], F32, kind="ExternalInput")
    # bqh/bkh: [128, nheads*2]  col h*2+c holds bias[c*128 + p]
    bqh_d = nc.dram_tensor("bqh", [128, nheads * 2], F32, kind="ExternalInput")
    bkh_d = nc.dram_tensor("bkh", [128, nheads * 2], F32, kind="ExternalInput")
    bv_d = nc.dram_tensor("bv", [nheads, DH], F32, kind="ExternalInput")
    negl_d = nc.dram_tensor("neglam", [128, nheads], F32, kind="ExternalInput")
    gsc_d = nc.dram_tensor("gsc", [128, nheads * 2], F32, kind="ExternalInput")
    out_d = nc.dram_tensor("out", [seq, E], F32, kind="ExternalOutput")

    with tile.TileContext(nc) as tc:
        with (
            tc.tile_pool(name="consts", bufs=1) as consts,
            tc.tile_pool(name="xt", bufs=1) as xtp,
            tc.tile_pool(name="xin", bufs=4) as xinp,
            tc.tile_pool(name="wts", bufs=2) as wts,
            tc.tile_pool(name="qk", bufs=1) as qkp,
            tc.tile_pool(name="vp", bufs=1) as vp,
            tc.tile_pool(name="e12", bufs=min(nsc + 2, 18)) as e12p,
            tc.tile_pool(name="oc", bufs=min(nt128 + 1, 17)) as ocp,
            tc.tile_pool(name="onp", bufs=4) as onp,
            tc.tile_pool(name="small", bufs=8) as small,
            tc.tile_pool(name="stats", bufs=2) as statsp,
            tc.tile_pool(name="outp", bufs=1) as outp,
            tc.tile_pool(name="ps_s", bufs=2, space="PSUM") as ps_s,
            tc.tile_pool(name="ps_o", bufs=4, space="PSUM") as ps_o,
            tc.tile_pool(name="ps_m", bufs=2, space="PSUM") as ps_m,
        ):
            ident = consts.tile([128, 128], F32)
            make_identity(nc, ident)
            eps_sb = consts.tile([128, 1], F32, tag="eps")
            nc.vector.memset(eps_sb, EPS)
            bqh_sb = consts.tile([128, nheads * 2], F32, tag="bqh")
            bkh_sb = consts.tile([128, nheads * 2], F32, tag="bkh")
            negl_sb = consts.tile([128, nheads], F32, tag="negl")
            gsc_sb = consts.tile([128, nheads * 2], F32, tag="gsc")
            nc.sync.dma_start(out=bqh_sb, in_=bqh_d[:, :])
            nc.sync.dma_start(out=bkh_sb, in_=bkh_d[:, :])
            nc.sync.dma_start(out=negl_sb, in_=negl_d[:, :])
            nc.sync.dma_start(out=gsc_sb, in_=gsc_d[:, :])

            # ---- transpose x, enc into [e, t] layout (2 e-chunks) ----
            xT = xtp.tile([128, 2, seq], F32R, tag="xT")
            encT = xtp.tile([128, 2, seq], F32R, tag="encT")
            for src, dst in ((x_d, xT), (enc_d, encT)):
                for n in range(seq // 128):
                    xin = xinp.tile([128, E], F32)
                    nc.sync.dma_start(out=xin, in_=src[n * 128:(n + 1) * 128, :])
                    for c in range(2):
                        pt = ps_m.tile([128, 512], F32, tag="mm")
                        nc.tensor.transpose(pt[:, 0:128],
                                            xin[:, c * 128:(c + 1) * 128], ident)
                        nc.vector.tensor_copy(dst[:, c, n * 128:(n + 1) * 128],
                                              pt[:, 0:128])

            outacc = outp.tile([128, nt128, E], F32, tag="outacc")

            for h in range(nheads):
                # ---- load weights for this head ----
                wq_sb = wts.tile([128, 2, DH], F32R, tag="wq")
                wk_sb = wts.tile([128, 2, DH], F32R, tag="wk")
                wv_sb = wts.tile([128, 2, DH], F32R, tag="wv")
                wo_sb = wts.tile([128, 2, E], F32R, tag="wo")
                nc.sync.dma_start(
                    out=wq_sb,
                    in_=wq_d[h].rearrange("(c p) d -> p c d", p=128).bitcast(F32R))
                nc.sync.dma_start(
                    out=wk_sb,
                    in_=wk_d[h].rearrange("(c p) d -> p c d", p=128).bitcast(F32R))
                nc.sync.dma_start(
                    out=wv_sb,
                    in_=wv_d[h].rearrange("(c p) d -> p c d", p=128).bitcast(F32R))
                nc.sync.dma_start(
                    out=wo_sb,
                    in_=wo_d[h].rearrange("(c p) e -> p c e", p=128).bitcast(F32R))
                # bv broadcast tile [128, DH]
                bvb = wts.tile([128, DH], F32, tag="bvb")
                bv_b = bass.AP(tensor=bv_d.ap().tensor, offset=h * DH,
                               ap=[[0, 128], [1, DH]])
                nc.gpsimd.dma_start(out=bvb, in_=bv_b)

                # ---- qT, kT projections: [d, t] ----
                qT = qkp.tile([128, 2, seq], F32R, tag="qT")
                kT = qkp.tile([128, 2, seq], F32R, tag="kT")
                for wsb, src, dst, bias_sb in ((wq_sb, xT, qT, bqh_sb),
                                               (wk_sb, encT, kT, bkh_sb)):
                    for m in range(2):
                        for tb in range(nt512):
                            ts = slice(tb * 512, (tb + 1) * 512)
                            ps = ps_m.tile([128, 512], F32, tag="mm")
                            nc.tensor.matmul(ps, wsb[:, 0, m * 128:(m + 1) * 128],
                                             src[:, 0, ts], start=True, stop=False)
                            nc.tensor.matmul(ps, wsb[:, 1, m * 128:(m + 1) * 128],
                                             src[:, 1, ts], start=False, stop=True)
                            nc.scalar.activation(dst[:, m, ts], ps, AF.Identity,
                                                 bias=bias_sb[:, h * 2 + m:h * 2 + m + 1],
                                                 scale=1.0)

                # ---- v_aug: [s, 257] per s-chunk ----
                v_sb = vp.tile([128, nsc, DH + 2], F32R, tag="v")
                for s2 in range(nsc // 2):
                    ps = ps_m.tile([128, 512], F32, tag="mm")
                    for half in range(2):
                        s = s2 * 2 + half
                        o_sl = ps[:, half * 256:(half + 1) * 256]
                        nc.tensor.matmul(o_sl, encT[:, 0, s * 128:(s + 1) * 128],
                                         wv_sb[:, 0, :], start=True, stop=False)
                        nc.tensor.matmul(o_sl, encT[:, 1, s * 128:(s + 1) * 128],
                                         wv_sb[:, 1, :], start=False, stop=True)
                    # v + bv  (bv broadcast along partitions)
                    for half in range(2):
                        s = s2 * 2 + half
                        nc.vector.scalar_tensor_tensor(
                            out=v_sb[:, s, 0:DH],
                            in0=ps[:, half * 256:(half + 1) * 256],
                            scalar=1.0, in1=bvb,
                            op0=ALU.mult, op1=ALU.add)
                nc.vector.memset(v_sb[:, :, DH:DH + 2].bitcast(F32), 1.0)

                ssq = statsp.tile([128, nt128], F32, tag="ssq")
                oc_tiles = {}

                # ---- attention over t-blocks ----
                for tb in range(ntb):
                    tsl = slice(tb * TBLK, (tb + 1) * TBLK)
                    e12s = []
                    for s in range(nsc):
                        pss = ps_s.tile([128, 2 * TBLK], F32, tag="s12")
                        nc.tensor.matmul(pss[:, 0:TBLK],
                                         kT[:, 0, s * 128:(s + 1) * 128],
                                         qT[:, 0, tsl], start=True, stop=True)
                        nc.tensor.matmul(pss[:, TBLK:2 * TBLK],
                                         kT[:, 1, s * 128:(s + 1) * 128],
                                         qT[:, 1, tsl], start=True, stop=True)
                        e12 = e12p.tile([128, 2 * TBLK], F32R, tag="e12")
                        nc.scalar.activation(e12, pss, AF.Exp, scale=SCALE)
                        e12s.append(e12)

                    for tc in range(TBLK // 128):
                        t128 = tb * (TBLK // 128) + tc
                        po1 = ps_o.tile([128, DH + 2], F32, tag="o")
                        po2 = ps_o.tile([128, DH + 2], F32, tag="o")
                        for s in range(nsc):
                            nc.tensor.matmul(po1,
                                             e12s[s][:, tc * 128:(tc + 1) * 128],
                                             v_sb[:, s, :],
                                             start=(s == 0), stop=(s == nsc - 1))
                        for s in range(nsc):
                            nc.tensor.matmul(po2,
                                             e12s[s][:, TBLK + tc * 128:TBLK + (tc + 1) * 128],
                                             v_sb[:, s, :],
                                             start=(s == 0), stop=(s == nsc - 1))
                        # combine: oc = o1/Z1 - lam*o2/Z2
                        rz1 = small.tile([128, 1], F32, tag="rz1")
                        rz2 = small.tile([128, 1], F32, tag="rz2")
                        nb = small.tile([128, 1], F32, tag="nb")
                        nc.vector.reciprocal(rz1, po1[:, DH:DH + 1])
                        nc.vector.reciprocal(rz2, po2[:, DH:DH + 1])
                        nc.vector.tensor_scalar_mul(nb, rz2, negl_sb[:, h:h + 1])
                        oc = ocp.tile([128, DH], F32, tag="oc")
                        nc.vector.tensor_scalar_mul(oc, po1[:, 0:DH], rz1)
                        nc.vector.scalar_tensor_tensor(
                            out=oc, in0=po2[:, 0:DH], scalar=nb, in1=oc,
                            op0=ALU.mult, op1=ALU.add)
                        osq = small.tile([128, DH], F32, tag="osq")
                        nc.vector.scalar_tensor_tensor(
                            out=osq, in0=oc, scalar=1.0, in1=oc,
                            op0=ALU.mult, op1=ALU.mult,
                            accum_out=ssq[:, t128:t128 + 1])
                        oc_tiles[t128] = oc

                # ---- rms + epilogue ----
                lnt = statsp.tile([128, nt128], F32, tag="lnt")
                rinv = statsp.tile([128, nt128], F32, tag="rinv")
                nc.scalar.activation(lnt, ssq, AF.Ln, bias=eps_sb, scale=1.0 / DH)
                nc.scalar.activation(rinv, lnt, AF.Exp, scale=-0.5)
                for t128 in range(nt128):
                    oc = oc_tiles.pop(t128)
                    nc.vector.tensor_scalar_mul(oc, oc, rinv[:, t128:t128 + 1])
                    onT = onp.tile([128, 2, 128], F32R, tag="onT")
                    for c in range(2):
                        pt = ps_m.tile([128, 512], F32, tag="mm")
                        nc.tensor.transpose(pt[:, 0:128],
                                            oc[:, c * 128:(c + 1) * 128], ident)
                        nc.scalar.mul(onT[:, c, :], pt[:, 0:128],
                                      gsc_sb[:, h * 2 + c:h * 2 + c + 1])
                    pw = ps_m.tile([128, 512], F32, tag="mm")
                    nc.tensor.matmul(pw[:, 0:E], onT[:, 0, :], wo_sb[:, 0, :],
                                     start=True, stop=False)
                    nc.tensor.matmul(pw[:, 0:E], onT[:, 1, :], wo_sb[:, 1, :],
                                     start=False, stop=True)
                    if h == 0:
                        nc.vector.tensor_copy(outacc[:, t128, :], pw[:, 0:E])
                    else:
                        nc.vector.tensor_add(outacc[:, t128, :],
                                             outacc[:, t128, :], pw[:, 0:E])

            nc.sync.dma_start(
                out=out_d.ap().rearrange("(n p) e -> p n e", p=128),
                in_=outacc)

    nc.compile()
    return nc


_PROGRAM_CACHE = {}


def _get_program(seq=T, nheads=HPC):
    key = (seq, nheads)
    if key not in _PROGRAM_CACHE:
        _PROGRAM_CACHE[key] = build_program(seq, nheads)
    return _PROGRAM_CACHE[key]


def make_in_maps(x, encoder_out, Wq, bq, Wk, bk, Wv, bv,
                 lq1, lk1, lq2, lk2, lam_init, g, Wo, bo):
    """Host-side sharding: build the 8 per-core input dicts."""
    f32 = np.float32
    x = np.asarray(x, f32)
    encoder_out = np.asarray(encoder_out, f32)
    Wq = np.asarray(Wq, f32); Wk = np.asarray(Wk, f32); Wv = np.asarray(Wv, f32)
    bq = np.asarray(bq, f32); bk = np.asarray(bk, f32); bv = np.asarray(bv, f32)
    Wo = np.asarray(Wo, f32)
    g = np.asarray(g, f32)
    lam = (np.exp(np.sum(np.asarray(lq1, f32) * np.asarray(lk1, f32), axis=-1))
           - np.exp(np.sum(np.asarray(lq2, f32) * np.asarray(lk2, f32), axis=-1))
           + np.asarray(lam_init, f32)).astype(f32)            # [H]
    Wo_h = Wo.reshape(H, DH, E)                                 # [H, DH, E]

    in_maps = []
    for core in range(N_CORES):
        b = core // 4
        hg = core % 4
        hs = slice(hg * HPC, (hg + 1) * HPC)
        heads = range(hg * HPC, (hg + 1) * HPC)
        bqh = np.empty((128, HPC * 2), f32)
        bkh = np.empty((128, HPC * 2), f32)
        gsc = np.empty((128, HPC * 2), f32)
        for i, hh in enumerate(heads):
            for c in range(2):
                bqh[:, i * 2 + c] = bq[hh, c * 128:(c + 1) * 128]
                bkh[:, i * 2 + c] = bk[hh, c * 128:(c + 1) * 128]
                gsc[:, i * 2 + c] = (1.0 - LAMBDA_INIT) * g[hh, c * 128:(c + 1) * 128]
        neglam = np.tile(-lam[hs], (128, 1)).astype(f32)        # [128, HPC]
        in_maps.append({
            "x": np.ascontiguousarray(x[b]),
            "enc": np.ascontiguousarray(encoder_out[b]),
            "wq": np.ascontiguousarray(Wq[hs]),
            "wk": np.ascontiguousarray(Wk[hs]),
            "wv": np.ascontiguousarray(Wv[hs]),
            "wo": np.ascontiguousarray(Wo_h[hs]),
            "bqh": bqh, "bkh": bkh,
            "bv": np.ascontiguousarray(bv[hs]),
            "neglam": neglam, "gsc": gsc,
        })
    return in_maps


def unshard(results, bo):
    """Sum row-parallel partials per batch, add bo."""
    bo = np.asarray(bo, np.float32)
    out = np.empty((B, T, E), np.float32)
    for b in range(B):
        acc = results[b * 4]["out"].astype(np.float32)
        for i in range(1, 4):
            acc = acc + results[b * 4 + i]["out"]
        out[b] = acc + bo[None, :]
    return out


def kernel(**inputs):
    nc = _get_program()
    in_maps = make_in_maps(**inputs)
    res = run_bass_kernel_spmd(nc, in_maps, list(range(N_CORES)))
    return unshard(res.results, inputs["bo"])


if __name__ == "__main__":
    build_program()
    print("program built ok")
